# revision 1
# baseline (speedup 1.0000x reference)
"""Depth-upsample module kernel for 8 TRN2 NeuronCores.

Pipeline per core (1/8 of batch*height):
  conv1 3x3 8->8 + bias + relu   (PE banded-dy matmuls, 3 per block)
  conv2 1x1 8->36 (raw)          (PE, 1 matmul per subpixel ab)
  E = exp(0.25*conv2 + 0.25*b2)  (ACT, PSUM->SBUF bf16)
  P = E * unfolded-depth         (DVE bf16)
  Num/Den = sum over 9 taps      (PE banded-ones matmuls -> PSUM quadrants)
  out = Num * recip(Den)         (DVE), strided DMA out (2x upsample interleave)

Layout: row-blocks of R=14 output rows; SBUF partitions pack (row, channel):
  conv input  xb   [(r16,i8)=128, 642]
  conv1 out   Y    [(r14,o8)=112, 640]
  conv2/E/P        [(r14,k9)=126, 640]  one tile per ab=2a+b
  Num/Den psum     [128, 640] quadrant ab holds rows 32*ab..+14
"""

import numpy as np
import ml_dtypes

H, W = 512, 640
N_IMG, C_IN = 4, 8
HALF = H // 2           # rows per core (shard = image x half)
RB = 14                 # output rows per block
WP = W + 2              # padded width
CW_CONST = 859          # packed const columns: w1(336) w2(504) band(14) b1(1) b2(4)


def _build_consts(conv1_w, conv1_b, conv2_w, conv2_b):
    f32 = np.float32
    # lhsT1[dx, (r,i), (r',o)] = W1[o,i,r-r',dx] for r-r' in {0,1,2}
    lhsT1 = np.zeros((3, 128, 112), f32)
    for dx in range(3):
        for rp in range(14):
            for dy in range(3):
                r = rp + dy
                lhsT1[dx, r * 8:(r + 1) * 8, rp * 8:(rp + 1) * 8] = \
                    conv1_w[:, :, dy, dx].T  # [i, o]
    # lhsT2[ab, (r,i), (r,k)] = W2[4k+ab, i]
    lhsT2 = np.zeros((4, 112, 126), f32)
    w2 = conv2_w[:, :, 0, 0]  # [36, 8]
    for ab in range(4):
        for r in range(14):
            for k in range(9):
                lhsT2[ab, r * 8:(r + 1) * 8, r * 9 + k] = w2[k * 4 + ab, :]
    # band[(r,k), r'] = 1 iff r == r'
    band = np.zeros((126, 14), ml_dtypes.bfloat16)
    for r in range(14):
        band[r * 9:(r + 1) * 9, r] = 1
    b1v = np.tile(conv1_b.astype(f32), 14)[:, None]            # [112,1]
    b2v = np.zeros((4, 126, 1), f32)
    for ab in range(4):
        for r in range(14):
            for k in range(9):
                b2v[ab, r * 9 + k, 0] = 0.25 * float(conv2_b[k * 4 + ab])
    return lhsT1, lhsT2, band, b1v, b2v


def _pack_consts(lhsT1, lhsT2, band, b1v, b2v):
    cst = np.zeros((128, CW_CONST), np.float32)
    for dx in range(3):
        cst[:, 112 * dx: 112 * (dx + 1)] = lhsT1[dx]
    for ab in range(4):
        cst[:112, 336 + 126 * ab: 336 + 126 * (ab + 1)] = lhsT2[ab]
    cst[:126, 840:854] = band.astype(np.float32)
    cst[:112, 854:855] = b1v
    for ab in range(4):
        cst[:126, 855 + ab: 856 + ab] = b2v[ab]
    return cst


def _build_bass():
    import concourse.bass as bass
    import concourse.bacc as bacc
    import concourse.tile as tile
    from concourse import mybir

    f32 = mybir.dt.float32
    bf16 = mybir.dt.bfloat16
    nc = bacc.Bacc(None, target_bir_lowering=False)

    X = nc.dram_tensor("xh", [C_IN, HALF + 2, WP], f32, kind="ExternalInput")
    DUNF = nc.dram_tensor("dunf", [HALF * 9, W], bf16, kind="ExternalInput")
    CONST = nc.dram_tensor("consts", [128, CW_CONST], f32, kind="ExternalInput")
    OUT = nc.dram_tensor("out", [2 * HALF, 2 * W], f32, kind="ExternalOutput")

    nblocks = (HALF + RB - 1) // RB  # 19 (last block R=4)

    with tile.TileContext(nc) as tc:
        with (
            tc.tile_pool(name="consts", bufs=1) as consts,
            tc.tile_pool(name="xp", bufs=3) as xp,
            tc.tile_pool(name="dp", bufs=3) as dp,
            tc.tile_pool(name="yp", bufs=2) as yp,
            tc.tile_pool(name="ep", bufs=6) as ep,
            tc.tile_pool(name="pp", bufs=6) as pp,
            tc.tile_pool(name="op", bufs=3) as op,
            tc.tile_pool(name="scr", bufs=2) as scr,
            tc.tile_pool(name="ps1", bufs=1, space="PSUM") as ps1,
            tc.tile_pool(name="ps2", bufs=1, space="PSUM") as ps2,
            tc.tile_pool(name="psn", bufs=1, space="PSUM") as psn,
            tc.tile_pool(name="psd", bufs=1, space="PSUM") as psd,
        ):
            cst = consts.tile([128, CW_CONST], f32, tag="cst")
            nc.sync.dma_start(out=cst, in_=CONST[:])
            w1t = [cst[:, 112 * dx: 112 * (dx + 1)] for dx in range(3)]
            w2t = [cst[:112, 336 + 126 * ab: 336 + 126 * (ab + 1)]
                   for ab in range(4)]
            band_f = cst[:126, 840:854]
            b1t = cst[:112, 854:855]
            b2t = [cst[:126, 855 + ab: 856 + ab] for ab in range(4)]
            bandt = consts.tile([126, 14], bf16, tag="bandbf")
            nc.vector.tensor_copy(bandt, band_f)
            # consume the const-DMA tick on PE (keeps real matmuls at <=1 wait)
            nc.tensor.ldweights(cst[:1, :2].bitcast(bf16))

            for b in range(nblocks):
                R = min(RB, HALF - RB * b)
                Rin = R + 2
                s = RB * b
                kp = R * 9   # partitions in (r,k) tiles
                yq = R * 8   # partitions in (r,o) tiles

                # --- load conv input block [(r,i), w] ---
                xb = xp.tile([128, WP], f32, tag="xb")
                x_in = bass.AP(
                    tensor=X[:].tensor, offset=s * WP,
                    ap=[[WP, Rin], [(HALF + 2) * WP, C_IN], [1, WP]],
                )
                nc.sync.dma_start(out=xb[: Rin * 8], in_=x_in)

                # --- load unfolded depth [(r,k), x] bf16 (host-prepared) ---
                dunf = dp.tile([126, W], bf16, tag="dunf")
                nc.sync.dma_start(out=dunf[:kp], in_=DUNF[9 * s: 9 * s + kp])
                scrap = scr.tile([1, 1], bf16, tag="scrap")
                nc.vector.tensor_copy(scrap, dunf[:1, :1])  # eat DMA tick

                # --- conv1: 3 dx matmuls x 2 col chunks -> psum1 ---
                nc.tensor.ldweights(xb[:1, :2].bitcast(bf16))  # eat DMA tick
                psum1 = ps1.tile([128, W], f32, tag="psum1")
                for c0, cn in ((0, 512), (512, 128)):
                    for dx in range(3):
                        nc.tensor.matmul(
                            psum1[:yq, c0:c0 + cn],
                            w1t[dx][: Rin * 8, :yq],
                            xb[: Rin * 8, dx + c0: dx + c0 + cn],
                            start=(dx == 0), stop=(dx == 2),
                        )

                # --- bias+relu -> Y (SBUF f32) ---
                Y = yp.tile([112, W], f32, tag="y")
                nc.scalar.activation(
                    out=Y[:yq], in_=psum1[:yq],
                    func=mybir.ActivationFunctionType.Relu,
                    bias=b1t[:yq], scale=1.0,
                )
                nc.tensor.ldweights(Y[:1, :2].bitcast(bf16))  # eat ACT tick

                # --- conv2 + exp per ab; then product with depth ---
                psumN = psn.tile([128, W], f32, tag="psumn")
                psumD = psd.tile([128, W], f32, tag="psumd")
                for ab in range(4):
                    psum2 = ps2.tile([128, W], f32, tag="psum2")
                    for c0, cn in ((0, 512), (512, 128)):
                        nc.tensor.matmul(
                            psum2[:kp, c0:c0 + cn],
                            w2t[ab][:yq, :kp],
                            Y[:yq, c0:c0 + cn],
                            start=True, stop=True,
                        )
                    E = ep.tile([126, W], bf16, tag="e")
                    nc.scalar.activation(
                        out=E[:kp], in_=psum2[:kp],
                        func=mybir.ActivationFunctionType.Exp,
                        bias=b2t[ab][:kp], scale=0.25,
                    )
                    P = pp.tile([126, W], bf16, tag="p")
                    nc.vector.tensor_mul(P[:kp], E[:kp], dunf[:kp])
                    # reduction over 9 taps -> psum quadrant ab
                    for c0, cn in ((0, 512), (512, 128)):
                        nc.tensor.matmul(
                            psumN[32 * ab: 32 * ab + R, c0:c0 + cn],
                            bandt[:kp, :R], P[:kp, c0:c0 + cn],
                            start=True, stop=True,
                            tile_position=(0, 32 * ab),
                        )
                        nc.tensor.matmul(
                            psumD[32 * ab: 32 * ab + R, c0:c0 + cn],
                            bandt[:kp, :R], E[:kp, c0:c0 + cn],
                            start=True, stop=True,
                            tile_position=(0, 32 * ab),
                        )

                # --- divide ---
                RD = op.tile([128, W], f32, tag="rd")
                nc.vector.reciprocal(out=RD, in_=psumD[:])
                O = op.tile([128, W], f32, tag="o")
                nc.vector.tensor_mul(O, psumN[:], RD)

                # --- interleaved store: out[2(s+r)+a, 2x+b] = O[32(2a+b)+r, x]
                for ab in range(4):
                    a, bb = ab >> 1, ab & 1
                    o_out = bass.AP(
                        tensor=OUT[:].tensor,
                        offset=(2 * s + a) * (2 * W) + bb,
                        ap=[[4 * W, R], [2, W]],
                    )
                    nc.sync.dma_start(out=o_out, in_=O[32 * ab: 32 * ab + R])

    nc.compile()
    return nc


_NC_CACHE = None


def prep_inputs(depth, cost_volume, conv1_w, conv1_b, conv2_w, conv2_b):
    depth = np.asarray(depth, np.float32)
    cv = np.asarray(cost_volume, np.float32).reshape(N_IMG, C_IN, H, W)
    lhsT1, lhsT2, band, b1v, b2v = _build_consts(
        np.asarray(conv1_w, np.float32), np.asarray(conv1_b, np.float32),
        np.asarray(conv2_w, np.float32), np.asarray(conv2_b, np.float32))
    cstpk = _pack_consts(lhsT1, lhsT2, band, b1v, b2v)

    # halo'd, zero-padded shards: core c = 2*n + h
    sw = np.lib.stride_tricks.sliding_window_view
    in_maps = []
    for n in range(N_IMG):
        cvp = np.zeros((C_IN, H + 2, WP), np.float32)
        cvp[:, 1:H + 1, 1:W + 1] = cv[n]
        dpad = np.zeros((H + 2, WP), np.float32)
        dpad[1:H + 1, 1:W + 1] = depth[n]
        # unfold: du[(r*9 + ky*3 + kx), x] = dpad[r+ky, x+kx]
        win = sw(dpad, (3, W + 2))[:H, 0]                # [H,3,W+2]
        du = np.stack([win[:, :, kx:kx + W] for kx in range(3)], 2)
        du = du.reshape(H * 9, W).astype(ml_dtypes.bfloat16)
        for h in range(2):
            r0 = h * HALF
            in_maps.append({
                "xh": np.ascontiguousarray(cvp[:, r0:r0 + HALF + 2, :]),
                "dunf": np.ascontiguousarray(du[9 * r0: 9 * (r0 + HALF)]),
                "consts": cstpk,
            })
    return in_maps


def kernel(depth, cost_volume, conv1_w, conv1_b, conv2_w, conv2_b):
    global _NC_CACHE
    from concourse.bass_utils import run_bass_kernel_spmd

    in_maps = prep_inputs(depth, cost_volume, conv1_w, conv1_b,
                          conv2_w, conv2_b)
    if _NC_CACHE is None:
        _NC_CACHE = _build_bass()
    res = run_bass_kernel_spmd(_NC_CACHE, in_maps, core_ids=list(range(8)))
    out = np.empty((N_IMG, 2 * H, 2 * W), np.float32)
    for c, r in enumerate(res.results):
        n, h = c // 2, c % 2
        out[n, 2 * h * HALF: 2 * (h + 1) * HALF, :] = r["out"]
    return out



# revision 4
# speedup vs baseline: 20.0878x; 20.0878x over previous
"""Depth-upsample module kernel for 8 TRN2 NeuronCores.

Pipeline per core (1/8 of batch*height), all matmuls bf16:
  conv1 3x3 8->8 + bias + relu   (PE banded-dy matmuls, 3 dx passes)
  conv2 1x1 8->36 (raw)          (PE, per subpixel ab=2a+b)
  E = exp(0.25*conv2 + 0.25*b2)  (ACT, PSUM->SBUF bf16, cols 640:1280 of PEt)
  P = E * unfolded-depth         (DVE bf16, cols 0:640 of PEt)
  Num/Den = sum over 9 taps      (PE banded-ones matmuls over the P||E tile:
                                  3 col-chunks/ab -> psumND[quadrant 32ab,
                                  Num cols 0:640 | Den cols 640:1280])
  RD = ~1/Den                    (DVE reciprocal_approx_fast)
  OI[r, 2x+b] = Num*RD           (2 DVE muls w/ stride-2 col writes: the 2x
                                  upsample b-interleave happens in SBUF)
  out rows 2(s+r)+a              (2 contiguous-row DMAs per block)

Weights are zero-padded to 128 free cols (enables FWL; padded band lhsT
cols make non-quadrant psum partitions accumulate exact 0 across ab).

Layout: row-blocks of R=14 output rows; SBUF partitions pack (row, channel):
  conv input  xb   [(r16,i8)=128, 642] bf16
  conv1 out   Y    [(r14,o8)=112, 640] bf16
  P||E  PEt        [(r14,k9)=126, 1280] bf16, P=0:640, E=640:1280
  psumND           [128, 1280] quadrant ab rows 32ab..+14
  OI               [128, 1280] f32: rows 0..13 (a=0), 64..77 (a=1)
"""

import numpy as np
import ml_dtypes

H, W = 512, 640
N_IMG, C_IN = 4, 8
HALF = H // 2           # rows per core (shard = image x half)
RB = 14                 # output rows per block
WP = W + 2              # padded width
CWB = 11 * 128          # bf16 const cols: w1(3) w2(4) band(4), each 128


def _build_consts(conv1_w, conv1_b, conv2_w, conv2_b):
    bf16 = ml_dtypes.bfloat16
    # w1t[dx][8(rp+dy)+i, 8rp+o] = W1[o,i,dy,dx]   (banded over dy)
    w1t = np.zeros((3, 128, 128), bf16)
    for dx in range(3):
        for rp in range(14):
            for dy in range(3):
                r = rp + dy
                w1t[dx, r * 8:(r + 1) * 8, rp * 8:(rp + 1) * 8] = \
                    conv1_w[:, :, dy, dx].T  # [i, o]
    # w2t[ab][8r+i, 9r+k] = W2[4k+ab, i]
    w2t = np.zeros((4, 112, 128), bf16)
    w2 = conv2_w[:, :, 0, 0]  # [36, 8]
    for ab in range(4):
        for r in range(14):
            for k in range(9):
                w2t[ab, r * 8:(r + 1) * 8, r * 9 + k] = w2[k * 4 + ab, :]
    # bandT[ab][9r+k, 32ab+r] = 1  (tap-sum into quadrant ab; other cols 0)
    bandT = np.zeros((4, 126, 128), bf16)
    for ab in range(4):
        for r in range(14):
            bandT[ab, r * 9:(r + 1) * 9, 32 * ab + r] = 1
    b1v = np.tile(conv1_b.astype(np.float32), 14)[:, None]     # [112,1]
    b2v = np.zeros((4, 126, 1), np.float32)
    for ab in range(4):
        for r in range(14):
            for k in range(9):
                b2v[ab, r * 9 + k, 0] = 0.25 * float(conv2_b[k * 4 + ab])
    return w1t, w2t, bandT, b1v, b2v


def _pack_consts(w1t, w2t, bandT, b1v, b2v):
    cstb = np.zeros((128, CWB), ml_dtypes.bfloat16)
    for dx in range(3):
        cstb[:, 128 * dx: 128 * (dx + 1)] = w1t[dx]
    for ab in range(4):
        cstb[:112, 384 + 128 * ab: 384 + 128 * (ab + 1)] = w2t[ab]
        cstb[:126, 896 + 128 * ab: 896 + 128 * (ab + 1)] = bandT[ab]
    cstf = np.zeros((128, 5), np.float32)
    cstf[:112, 0:1] = b1v
    for ab in range(4):
        cstf[:126, 1 + ab: 2 + ab] = b2v[ab]
    return cstb, cstf


def _build_bass():
    import concourse.bass as bass
    import concourse.bacc as bacc
    import concourse.tile as tile
    from concourse import mybir

    f32 = mybir.dt.float32
    bf16 = mybir.dt.bfloat16
    nc = bacc.Bacc(None, target_bir_lowering=False)

    X = nc.dram_tensor("xh", [C_IN, HALF + 2, WP], bf16, kind="ExternalInput")
    DUNF = nc.dram_tensor("dunf", [HALF * 9, W], bf16, kind="ExternalInput")
    CSTB = nc.dram_tensor("cstb", [128, CWB], bf16, kind="ExternalInput")
    CSTF = nc.dram_tensor("cstf", [128, 5], f32, kind="ExternalInput")
    OUT = nc.dram_tensor("out", [2 * HALF, 2 * W], f32, kind="ExternalOutput")

    nblocks = (HALF + RB - 1) // RB  # 19 (last block R=4)

    with tile.TileContext(nc) as tc:
        with (
            tc.tile_pool(name="consts", bufs=1) as consts,
            tc.tile_pool(name="xp", bufs=3) as xp,
            tc.tile_pool(name="dp", bufs=3) as dp,
            tc.tile_pool(name="yp", bufs=2) as yp,
            tc.tile_pool(name="pe", bufs=3) as pe,
            tc.tile_pool(name="rp", bufs=2) as rp,
            tc.tile_pool(name="op", bufs=3) as op,
            tc.tile_pool(name="scr", bufs=2) as scr,
            tc.tile_pool(name="ps1", bufs=1, space="PSUM") as ps1,
            tc.tile_pool(name="ps2", bufs=2, space="PSUM") as ps2,
            tc.tile_pool(name="psnd", bufs=1, space="PSUM") as psnd,
        ):
            cstb = consts.tile([128, CWB], bf16, tag="cstb")
            nc.sync.dma_start(out=cstb, in_=CSTB[:])
            cstf = consts.tile([128, 5], f32, tag="cstf")
            nc.sync.dma_start(out=cstf, in_=CSTF[:])
            w1t = [cstb[:, 128 * dx: 128 * (dx + 1)] for dx in range(3)]
            w2t = [cstb[:112, 384 + 128 * ab: 384 + 128 * (ab + 1)]
                   for ab in range(4)]
            bandT = [cstb[:126, 896 + 128 * ab: 896 + 128 * (ab + 1)]
                     for ab in range(4)]
            b1t = cstf[:112, 0:1]
            b2t = [cstf[:126, 1 + ab: 2 + ab] for ab in range(4)]
            # consume the const-DMA ticks (keeps real matmuls at <=1 wait)
            nc.tensor.ldweights(cstb[:1, :2])
            scrap0 = scr.tile([1, 1], f32, tag="scrap0")
            nc.vector.tensor_copy(scrap0, cstf[:1, :1])

            for b in range(nblocks):
                R = min(RB, HALF - RB * b)
                Rin = R + 2
                s = RB * b
                kp = R * 9   # partitions in (r,k) tiles
                yq = R * 8   # partitions in (r,o) tiles

                # --- load conv input block [(r,i), w] bf16 ---
                xb = xp.tile([128, WP], bf16, tag="xb")
                x_in = bass.AP(
                    tensor=X[:].tensor, offset=s * WP,
                    ap=[[WP, Rin], [(HALF + 2) * WP, C_IN], [1, WP]],
                )
                nc.sync.dma_start(out=xb[: Rin * 8], in_=x_in)

                # --- load unfolded depth [(r,k), x] bf16 (host-prepared) ---
                dunf = dp.tile([126, W], bf16, tag="dunf")
                nc.sync.dma_start(out=dunf[:kp], in_=DUNF[9 * s: 9 * s + kp])
                scrap = scr.tile([1, 1], bf16, tag="scrap")
                nc.vector.tensor_copy(scrap, dunf[:1, :1])  # eat DMA tick

                # --- conv1: 3 dx matmuls x 2 col chunks -> psum1 ---
                nc.tensor.ldweights(xb[:1, :2])  # eat DMA tick
                psum1 = ps1.tile([128, W], f32, tag="psum1")
                for dx in range(3):
                    for c0, cn in ((0, 512), (512, 128)):
                        nc.tensor.matmul(
                            psum1[:, c0:c0 + cn],
                            w1t[dx][: Rin * 8],
                            xb[: Rin * 8, dx + c0: dx + c0 + cn],
                            start=(dx == 0), stop=(dx == 2),
                        )

                # --- bias+relu -> Y (SBUF bf16) ---
                Y = yp.tile([112, W], bf16, tag="y")
                nc.scalar.activation(
                    out=Y[:yq], in_=psum1[:yq],
                    func=mybir.ActivationFunctionType.Relu,
                    bias=b1t[:yq], scale=1.0,
                )
                nc.tensor.ldweights(Y[:1, :2])  # eat ACT tick

                # --- conv2 + exp + P=E*d per ab; band-sum into psumND ---
                # psum2 is a single 1-bank [128,512] tile; conv2+exp run in
                # 512/128 col chunks so PSUM stays within 8 banks total.
                psumND = psnd.tile([128, 2 * W], f32, tag="psumnd")
                for ab in range(4):
                    PEt = pe.tile([126, 2 * W], bf16, tag="pet")
                    for c0, cn in ((0, 512), (512, 128)):
                        psum2 = ps2.tile([128, 512], f32, tag="psum2")
                        nc.tensor.matmul(
                            psum2[:, :cn],
                            w2t[ab][:yq],
                            Y[:yq, c0:c0 + cn],
                            start=True, stop=True,
                        )
                        nc.scalar.activation(
                            out=PEt[:kp, W + c0: W + c0 + cn],
                            in_=psum2[:kp, :cn],
                            func=mybir.ActivationFunctionType.Exp,
                            bias=b2t[ab][:kp], scale=0.25,
                        )
                    nc.vector.tensor_mul(PEt[:kp, :W], PEt[:kp, W:], dunf[:kp])
                    # tap reduction: Num -> cols 0:640, Den -> cols 640:1280
                    for c0, cn in ((0, 512), (512, 512), (1024, 256)):
                        nc.tensor.matmul(
                            psumND[:, c0:c0 + cn],
                            bandT[ab][:kp],
                            PEt[:kp, c0:c0 + cn],
                            start=(ab == 0), stop=(ab == 3),
                        )

                # --- divide + b-interleave into OI (strided DVE writes) ---
                RD = rp.tile([128, W], f32, tag="rd")
                nc.vector.reciprocal_approx_fast(out=RD, in_=psumND[:, W:])
                OI = op.tile([128, 2 * W], f32, tag="oi")
                nc.vector.tensor_mul(
                    OI[0:78, 0:2 * W:2], psumND[0:78, :W], RD[0:78])
                # odd cols: quadrant reads can't span >32 partitions from
                # base 32, so two 14-partition ops (bases 32 and 96)
                nc.vector.tensor_mul(
                    OI[0:R, 1:2 * W:2], psumND[32:32 + R, :W], RD[32:32 + R])
                nc.vector.tensor_mul(
                    OI[64:64 + R, 1:2 * W:2], psumND[96:96 + R, :W],
                    RD[96:96 + R])

                # --- contiguous-row stores: out[2(s+r)+a, :] ---
                for a in range(2):
                    o_out = bass.AP(
                        tensor=OUT[:].tensor,
                        offset=(2 * s + a) * (2 * W),
                        ap=[[4 * W, R], [1, 2 * W]],
                    )
                    nc.sync.dma_start(out=o_out, in_=OI[64 * a: 64 * a + R])

    nc.compile()
    return nc


_NC_CACHE = None


def prep_inputs(depth, cost_volume, conv1_w, conv1_b, conv2_w, conv2_b):
    bf16 = ml_dtypes.bfloat16
    depth = np.asarray(depth, np.float32)
    cv = np.asarray(cost_volume, np.float32).reshape(N_IMG, C_IN, H, W)
    w1t, w2t, bandT, b1v, b2v = _build_consts(
        np.asarray(conv1_w, np.float32), np.asarray(conv1_b, np.float32),
        np.asarray(conv2_w, np.float32), np.asarray(conv2_b, np.float32))
    cstb, cstf = _pack_consts(w1t, w2t, bandT, b1v, b2v)

    # halo'd, zero-padded shards: core c = 2*n + h
    sw = np.lib.stride_tricks.sliding_window_view
    in_maps = []
    for n in range(N_IMG):
        cvp = np.zeros((C_IN, H + 2, WP), bf16)
        cvp[:, 1:H + 1, 1:W + 1] = cv[n]
        dpad = np.zeros((H + 2, WP), np.float32)
        dpad[1:H + 1, 1:W + 1] = depth[n]
        # unfold: du[(r*9 + ky*3 + kx), x] = dpad[r+ky, x+kx]
        win = sw(dpad, (3, W + 2))[:H, 0]                # [H,3,W+2]
        du = np.stack([win[:, :, kx:kx + W] for kx in range(3)], 2)
        du = du.reshape(H * 9, W).astype(bf16)
        for h in range(2):
            r0 = h * HALF
            in_maps.append({
                "xh": np.ascontiguousarray(cvp[:, r0:r0 + HALF + 2, :]),
                "dunf": np.ascontiguousarray(du[9 * r0: 9 * (r0 + HALF)]),
                "cstb": cstb,
                "cstf": cstf,
            })
    return in_maps


def kernel(depth, cost_volume, conv1_w, conv1_b, conv2_w, conv2_b):
    global _NC_CACHE
    from concourse.bass_utils import run_bass_kernel_spmd

    in_maps = prep_inputs(depth, cost_volume, conv1_w, conv1_b,
                          conv2_w, conv2_b)
    if _NC_CACHE is None:
        _NC_CACHE = _build_bass()
    res = run_bass_kernel_spmd(_NC_CACHE, in_maps, core_ids=list(range(8)))
    out = np.empty((N_IMG, 2 * H, 2 * W), np.float32)
    for c, r in enumerate(res.results):
        n, h = c // 2, c % 2
        out[n, 2 * h * HALF: 2 * (h + 1) * HALF, :] = r["out"]
    return out


# revision 12
# speedup vs baseline: 21.6681x; 1.0787x over previous
"""Depth-upsample module kernel for 8 TRN2 NeuronCores.

Pipeline per core (1/8 of batch*height), all matmuls bf16:
  conv1 3x3 8->8 + bias + relu   (PE banded-dy matmuls, 3 dx passes)
  conv2 1x1 8->36 (raw)          (PE, per subpixel ab=2a+b)
  E = exp(0.25*conv2 + 0.25*b2)  (ACT, PSUM->SBUF bf16, cols 640:1280 of PEt)
  P = E * unfolded-depth         (DVE bf16, cols 0:640 of PEt)
  Num/Den = sum over 9 taps      (PE banded-ones matmuls over the P||E tile:
                                  3 col-chunks/ab -> psumND[quadrant 32ab,
                                  Num cols 0:640 | Den cols 640:1280])
  RD = ~1/Den                    (DVE reciprocal_approx_fast)
  OI[r, 2x+b] = Num*RD           (2 DVE muls w/ stride-2 col writes: the 2x
                                  upsample b-interleave happens in SBUF)
  out rows 2(s+r)+a              (2 contiguous-row DMAs per block)

Weights are zero-padded to 128 free cols (enables FWL; padded band lhsT
cols make non-quadrant psum partitions accumulate exact 0 across ab).

Layout: row-blocks of R=14 output rows; SBUF partitions pack (row, channel):
  conv input  xb   [(r16,i8)=128, 642] bf16
  conv1 out   Y    [(r14,o8)=112, 640] bf16
  P||E  PEt        [(r14,k9)=126, 1280] bf16, P=0:640, E=640:1280
  psumND           [128, 1280] quadrant ab rows 32ab..+14
  OI               [128, 1280] f32: rows 0..13 (a=0), 64..77 (a=1)
"""

import numpy as np
import ml_dtypes

H, W = 512, 640
N_IMG, C_IN = 4, 8
HALF = H // 2           # rows per core (shard = image x half)
RB = 14                 # output rows per block
WP = W + 2              # padded width
CWB = 11 * 128          # bf16 const cols: w1(3) w2(4) band(4), each 128


def _build_consts(conv1_w, conv1_b, conv2_w, conv2_b):
    bf16 = ml_dtypes.bfloat16
    # w1t[dx][8(rp+dy)+i, 8rp+o] = W1[o,i,dy,dx]   (banded over dy)
    w1t = np.zeros((3, 128, 128), bf16)
    for dx in range(3):
        for rp in range(14):
            for dy in range(3):
                r = rp + dy
                w1t[dx, r * 8:(r + 1) * 8, rp * 8:(rp + 1) * 8] = \
                    conv1_w[:, :, dy, dx].T  # [i, o]
    # w2t[ab][8r+i, 9r+k] = W2[4k+ab, i]   (rows padded to K=128 for FWL)
    w2t = np.zeros((4, 128, 128), bf16)
    w2 = conv2_w[:, :, 0, 0]  # [36, 8]
    for ab in range(4):
        for r in range(14):
            for k in range(9):
                w2t[ab, r * 8:(r + 1) * 8, r * 9 + k] = w2[k * 4 + ab, :]
    # bandT[ab][9r+k, 32ab+r] = 1  (tap-sum into quadrant ab; other cols 0)
    bandT = np.zeros((4, 128, 128), bf16)
    for ab in range(4):
        for r in range(14):
            bandT[ab, r * 9:(r + 1) * 9, 32 * ab + r] = 1
    b1v = np.tile(conv1_b.astype(np.float32), 14)[:, None]     # [112,1]
    b2v = np.zeros((4, 126, 1), np.float32)
    for ab in range(4):
        for r in range(14):
            for k in range(9):
                b2v[ab, r * 9 + k, 0] = 0.25 * float(conv2_b[k * 4 + ab])
    return w1t, w2t, bandT, b1v, b2v


def _pack_consts(w1t, w2t, bandT, b1v, b2v):
    cstb = np.zeros((128, CWB), ml_dtypes.bfloat16)
    for dx in range(3):
        cstb[:, 128 * dx: 128 * (dx + 1)] = w1t[dx]
    for ab in range(4):
        cstb[:, 384 + 128 * ab: 384 + 128 * (ab + 1)] = w2t[ab]
        cstb[:, 896 + 128 * ab: 896 + 128 * (ab + 1)] = bandT[ab]
    cstf = np.zeros((128, 5), np.float32)
    cstf[:112, 0:1] = b1v
    for ab in range(4):
        cstf[:126, 1 + ab: 2 + ab] = b2v[ab]
    return cstb, cstf


def _build_bass():
    import concourse.bass as bass
    import concourse.bacc as bacc
    import concourse.tile as tile
    from concourse import mybir

    f32 = mybir.dt.float32
    bf16 = mybir.dt.bfloat16
    nc = bacc.Bacc(None, target_bir_lowering=False)

    nblk = (HALF + RB - 1) // RB
    X = nc.dram_tensor("xh", [C_IN, HALF + 2, WP], bf16, kind="ExternalInput")
    # per-block 128-row tap pages (rows 9R..128 zeroed host-side): K=128
    # band/P-mul operands with no garbage partitions
    DUNF = nc.dram_tensor("dunf", [nblk * 128, W], bf16, kind="ExternalInput")
    CSTB = nc.dram_tensor("cstb", [128, CWB], bf16, kind="ExternalInput")
    CSTF = nc.dram_tensor("cstf", [128, 5], f32, kind="ExternalInput")
    OUT = nc.dram_tensor("out", [2 * HALF, 2 * W], f32, kind="ExternalOutput")

    nblocks = (HALF + RB - 1) // RB  # 19 (last block R=4)

    with tile.TileContext(nc) as tc:
        with (
            tc.tile_pool(name="consts", bufs=1) as consts,
            tc.tile_pool(name="xp", bufs=3) as xp,
            tc.tile_pool(name="dp", bufs=3) as dp,
            tc.tile_pool(name="yp", bufs=2) as yp,
            tc.tile_pool(name="pe", bufs=3) as pe,
            tc.tile_pool(name="rp", bufs=2) as rp,
            tc.tile_pool(name="op", bufs=3) as op,
            tc.tile_pool(name="scr", bufs=2) as scr,
            tc.tile_pool(name="ps1", bufs=1, space="PSUM") as ps1,
            tc.tile_pool(name="ps2", bufs=2, space="PSUM") as ps2,
            tc.tile_pool(name="psnd", bufs=1, space="PSUM") as psnd,
        ):
            cstb = consts.tile([128, CWB], bf16, tag="cstb")
            nc.sync.dma_start(out=cstb, in_=CSTB[:])
            cstf = consts.tile([128, 5], f32, tag="cstf")
            nc.sync.dma_start(out=cstf, in_=CSTF[:])
            w1t = [cstb[:, 128 * dx: 128 * (dx + 1)] for dx in range(3)]
            w2t = [cstb[:, 384 + 128 * ab: 384 + 128 * (ab + 1)]
                   for ab in range(4)]
            bandT = [cstb[:, 896 + 128 * ab: 896 + 128 * (ab + 1)]
                     for ab in range(4)]
            b1t = cstf[:, 0:1]
            b2t = [cstf[:, 1 + ab: 2 + ab] for ab in range(4)]
            # consume the const-DMA ticks (keeps real matmuls at <=1 wait)
            nc.tensor.ldweights(cstb[:1, :2])
            scrap0 = scr.tile([1, 1], f32, tag="scrap0")
            nc.vector.tensor_copy(scrap0, cstf[:1, :1])

            for b in range(nblocks):
                R = min(RB, HALF - RB * b)
                Rin = R + 2
                s = RB * b
                kp = R * 9   # partitions in (r,k) tiles
                yq = R * 8   # partitions in (r,o) tiles

                # --- load conv input block [(r,i), w] bf16 ---
                xb = xp.tile([128, WP], bf16, tag="xb")
                x_in = bass.AP(
                    tensor=X[:].tensor, offset=s * WP,
                    ap=[[WP, Rin], [(HALF + 2) * WP, C_IN], [1, WP]],
                )
                nc.sync.dma_start(out=xb[: Rin * 8], in_=x_in)

                # --- load unfolded depth [(r,k), x] bf16 (host-prepared) ---
                dunf = dp.tile([128, W], bf16, tag="dunf")
                nc.sync.dma_start(out=dunf, in_=DUNF[128 * b: 128 * (b + 1)])
                scrap = scr.tile([1, 1], bf16, tag="scrap")
                nc.vector.tensor_copy(scrap, dunf[:1, :1])  # eat DMA tick

                # --- conv1: 3 dx matmuls x 2 col chunks -> psum1 ---
                nc.tensor.ldweights(xb[:1, :2])  # eat DMA tick
                psum1 = ps1.tile([128, W], f32, tag="psum1")
                for dx in range(3):
                    for c0, cn in ((0, 512), (512, 128)):
                        nc.tensor.matmul(
                            psum1[:, c0:c0 + cn],
                            w1t[dx][: Rin * 8],
                            xb[: Rin * 8, dx + c0: dx + c0 + cn],
                            start=(dx == 0), stop=(dx == 2),
                        )

                # --- bias+relu -> Y (SBUF bf16, full 128 rows: padded rows
                # compute relu(0+0)=0 so conv2 can run K=128 / FWL) ---
                Y = yp.tile([128, W], bf16, tag="y")
                nc.scalar.activation(
                    out=Y, in_=psum1[:],
                    func=mybir.ActivationFunctionType.Relu,
                    bias=b1t, scale=1.0,
                )
                nc.tensor.ldweights(Y[:1, :2])  # eat ACT tick

                # --- conv2 + exp + P=E*d per ab; band-sum into psumND ---
                # psum2 is a single 1-bank [128,512] tile; conv2+exp run in
                # 512/128 col chunks so PSUM stays within 8 banks total.
                psumND = psnd.tile([128, 2 * W], f32, tag="psumnd")
                for ab in range(4):
                    PEt = pe.tile([128, 2 * W], bf16, tag="pet")
                    for c0, cn in ((0, 512), (512, 128)):
                        psum2 = ps2.tile([128, 512], f32, tag="psum2")
                        nc.tensor.matmul(
                            psum2[:, :cn],
                            w2t[ab],
                            Y[:, c0:c0 + cn],
                            start=True, stop=True,
                        )
                        nc.scalar.activation(
                            out=PEt[:, W + c0: W + c0 + cn],
                            in_=psum2[:, :cn],
                            func=mybir.ActivationFunctionType.Exp,
                            bias=b2t[ab], scale=0.25,
                        )
                    nc.vector.tensor_mul(PEt[:, :W], PEt[:, W:], dunf)
                    # tap reduction: Num -> cols 0:640, Den -> cols 640:1280
                    for c0, cn in ((0, 512), (512, 512), (1024, 256)):
                        nc.tensor.matmul(
                            psumND[:, c0:c0 + cn],
                            bandT[ab],
                            PEt[:, c0:c0 + cn],
                            start=(ab == 0), stop=(ab == 3),
                        )

                # --- divide + b-interleave into OI (strided DVE writes) ---
                RD = rp.tile([128, W], f32, tag="rd")
                nc.vector.reciprocal_approx_fast(out=RD, in_=psumND[:, W:])
                OI = op.tile([128, 2 * W], f32, tag="oi")
                nc.vector.tensor_mul(
                    OI[0:78, 0:2 * W:2], psumND[0:78, :W], RD[0:78])
                # odd cols: quadrant reads can't span >32 partitions from
                # base 32, so two 14-partition ops (bases 32 and 96)
                nc.vector.tensor_mul(
                    OI[0:R, 1:2 * W:2], psumND[32:32 + R, :W], RD[32:32 + R])
                nc.vector.tensor_mul(
                    OI[64:64 + R, 1:2 * W:2], psumND[96:96 + R, :W],
                    RD[96:96 + R])

                # --- contiguous-row stores: out[2(s+r)+a, :] ---
                for a in range(2):
                    o_out = bass.AP(
                        tensor=OUT[:].tensor,
                        offset=(2 * s + a) * (2 * W),
                        ap=[[4 * W, R], [1, 2 * W]],
                    )
                    nc.sync.dma_start(out=o_out, in_=OI[64 * a: 64 * a + R])

    nc.compile()
    return nc


_NC_CACHE = None


def prep_inputs(depth, cost_volume, conv1_w, conv1_b, conv2_w, conv2_b):
    bf16 = ml_dtypes.bfloat16
    depth = np.asarray(depth, np.float32)
    cv = np.asarray(cost_volume, np.float32).reshape(N_IMG, C_IN, H, W)
    w1t, w2t, bandT, b1v, b2v = _build_consts(
        np.asarray(conv1_w, np.float32), np.asarray(conv1_b, np.float32),
        np.asarray(conv2_w, np.float32), np.asarray(conv2_b, np.float32))
    cstb, cstf = _pack_consts(w1t, w2t, bandT, b1v, b2v)

    # halo'd, zero-padded shards: core c = 2*n + h
    sw = np.lib.stride_tricks.sliding_window_view
    in_maps = []
    for n in range(N_IMG):
        cvp = np.zeros((C_IN, H + 2, WP), bf16)
        cvp[:, 1:H + 1, 1:W + 1] = cv[n]
        dpad = np.zeros((H + 2, WP), np.float32)
        dpad[1:H + 1, 1:W + 1] = depth[n]
        # unfold: du[(r*9 + ky*3 + kx), x] = dpad[r+ky, x+kx]
        win = sw(dpad, (3, W + 2))[:H, 0]                # [H,3,W+2]
        du = np.stack([win[:, :, kx:kx + W] for kx in range(3)], 2)
        du = du.reshape(H * 9, W).astype(bf16)
        nblk = (HALF + RB - 1) // RB
        for h in range(2):
            r0 = h * HALF
            # per-block 128-row pages: rows 9R..128 zero
            dup = np.zeros((nblk * 128, W), bf16)
            for blk in range(nblk):
                R = min(RB, HALF - RB * blk)
                src = 9 * (r0 + RB * blk)
                dup[128 * blk: 128 * blk + 9 * R] = du[src: src + 9 * R]
            in_maps.append({
                "xh": np.ascontiguousarray(cvp[:, r0:r0 + HALF + 2, :]),
                "dunf": dup,
                "cstb": cstb,
                "cstf": cstf,
            })
    return in_maps


def kernel(depth, cost_volume, conv1_w, conv1_b, conv2_w, conv2_b):
    global _NC_CACHE
    from concourse.bass_utils import run_bass_kernel_spmd

    in_maps = prep_inputs(depth, cost_volume, conv1_w, conv1_b,
                          conv2_w, conv2_b)
    if _NC_CACHE is None:
        _NC_CACHE = _build_bass()
    res = run_bass_kernel_spmd(_NC_CACHE, in_maps, core_ids=list(range(8)))
    out = np.empty((N_IMG, 2 * H, 2 * W), np.float32)
    for c, r in enumerate(res.results):
        n, h = c // 2, c % 2
        out[n, 2 * h * HALF: 2 * (h + 1) * HALF, :] = r["out"]
    return out


# revision 22
# speedup vs baseline: 27.4255x; 1.2657x over previous
"""Depth-upsample module kernel for 8 TRN2 NeuronCores.

Pipeline per core (1/8 of batch*height), all matmuls bf16:
  conv1 3x3 8->8 + bias + relu   (PE banded-dy matmuls, 3 dx passes)
  conv2 1x1 8->36 (raw)          (PE, per subpixel ab=2a+b)
  E = exp(0.25*conv2 + 0.25*b2)  (ACT, PSUM->SBUF bf16, cols 640:1280 of PEt)
  P = E * unfolded-depth         (DVE bf16, cols 0:640 of PEt)
  Num/Den = sum over 9 taps      (PE banded-ones matmuls over the P||E tile:
                                  3 col-chunks/ab -> psumND[quadrant 32ab,
                                  Num cols 0:640 | Den cols 640:1280])
  RD = ~1/Den                    (DVE reciprocal_approx_fast)
  OI[r, 2x+b] = Num*RD           (2 DVE muls w/ stride-2 col writes: the 2x
                                  upsample b-interleave happens in SBUF)
  out rows 2(s+r)+a              (2 contiguous-row DMAs per block)

Weights are zero-padded to 128 free cols (enables FWL; padded band lhsT
cols make non-quadrant psum partitions accumulate exact 0 across ab).

Layout: row-blocks of R=14 output rows; SBUF partitions pack (row, channel):
  conv input  xb   [(r16,i8)=128, 642] bf16
  conv1 out   Y    [(r14,o8)=112, 640] bf16
  P||E  PEt        [(r14,k9)=126, 1280] bf16, P=0:640, E=640:1280
  psumND           [128, 1280] quadrant ab rows 32ab..+14
  OI               [128, 1280] f32: rows 0..13 (a=0), 64..77 (a=1)
"""

import numpy as np
import ml_dtypes


def _patch_ldw_opt():
    """walrus is invoked with --enable-ldw-opt=false hardcoded, which keeps
    every matmul's LDWEIGHTS on the slow path (~100ns serial each, ~28% of
    PE time here). Rewrite the flag on the walrus argv."""
    from concourse import bass_utils as _bu
    if getattr(_bu, "_ldw_patched", False):
        return
    _orig = _bu.run_command

    def _patched(argv, **kwargs):
        argv = ["--enable-ldw-opt=true" if a == "--enable-ldw-opt=false"
                else a for a in argv]
        return _orig(argv, **kwargs)

    _bu.run_command = _patched
    _bu._ldw_patched = True


import os as _os
if _os.environ.get("LDW_OPT", "0") == "1":
    _patch_ldw_opt()

H, W = 512, 640
N_IMG, C_IN = 4, 8
HALF = H // 2           # rows per core (shard = image x half)
RB = 14                 # output rows per block
WP = W + 2              # padded width
CWB = 11 * 128          # bf16 const cols: w1(3) w2(4) band(4), each 128


def _build_consts(conv1_w, conv1_b, conv2_w, conv2_b):
    bf16 = ml_dtypes.bfloat16
    # w1t[dx][8(rp+dy)+i, 8rp+o] = W1[o,i,dy,dx]   (banded over dy)
    w1t = np.zeros((3, 128, 128), bf16)
    for dx in range(3):
        for rp in range(14):
            for dy in range(3):
                r = rp + dy
                w1t[dx, r * 8:(r + 1) * 8, rp * 8:(rp + 1) * 8] = \
                    conv1_w[:, :, dy, dx].T  # [i, o]
    # w2t[ab][8r+i, 9r+k] = W2[4k+ab, i]   (rows padded to K=128 for FWL)
    w2t = np.zeros((4, 128, 128), bf16)
    w2 = conv2_w[:, :, 0, 0]  # [36, 8]
    for ab in range(4):
        for r in range(14):
            for k in range(9):
                w2t[ab, r * 8:(r + 1) * 8, r * 9 + k] = w2[k * 4 + ab, :]
    # bandT[ab][9r+k, 32ab+r] = 1  (tap-sum into quadrant ab; other cols 0)
    bandT = np.zeros((4, 128, 128), bf16)
    for ab in range(4):
        for r in range(14):
            bandT[ab, r * 9:(r + 1) * 9, 32 * ab + r] = 1
    b1v = np.tile(conv1_b.astype(np.float32), 14)[:, None]     # [112,1]
    b2v = np.zeros((4, 126, 1), np.float32)
    for ab in range(4):
        for r in range(14):
            for k in range(9):
                b2v[ab, r * 9 + k, 0] = 0.25 * float(conv2_b[k * 4 + ab])
    return w1t, w2t, bandT, b1v, b2v


def _pack_consts(w1t, w2t, bandT, b1v, b2v):
    cstb = np.zeros((128, CWB), ml_dtypes.bfloat16)
    for dx in range(3):
        cstb[:, 128 * dx: 128 * (dx + 1)] = w1t[dx]
    for ab in range(4):
        cstb[:, 384 + 128 * ab: 384 + 128 * (ab + 1)] = w2t[ab]
        cstb[:, 896 + 128 * ab: 896 + 128 * (ab + 1)] = bandT[ab]
    cstf = np.zeros((128, 5), np.float32)
    cstf[:112, 0:1] = b1v
    for ab in range(4):
        cstf[:126, 1 + ab: 2 + ab] = b2v[ab]
    return cstb, cstf


def _build_bass():
    import concourse.bass as bass
    import concourse.bacc as bacc
    import concourse.tile as tile
    from concourse import mybir

    f32 = mybir.dt.float32
    bf16 = mybir.dt.bfloat16
    nc = bacc.Bacc(None, target_bir_lowering=False)

    nblk = (HALF + RB - 1) // RB
    X = nc.dram_tensor("xh", [C_IN, HALF + 2, WP], bf16, kind="ExternalInput")
    # per-block 128-row tap pages (rows 9R..128 zeroed host-side): K=128
    # band/P-mul operands with no garbage partitions
    DUNF = nc.dram_tensor("dunf", [nblk * 128, W], bf16, kind="ExternalInput")
    CSTB = nc.dram_tensor("cstb", [128, CWB], bf16, kind="ExternalInput")
    CSTF = nc.dram_tensor("cstf", [128, 5], f32, kind="ExternalInput")
    OUT = nc.dram_tensor("out", [2 * HALF, 2 * W], f32, kind="ExternalOutput")

    nblocks = (HALF + RB - 1) // RB  # 19 (last block R=4)

    with tile.TileContext(nc) as tc:
        with (
            tc.tile_pool(name="consts", bufs=1) as consts,
            tc.tile_pool(name="xp", bufs=3) as xp,
            tc.tile_pool(name="dp", bufs=3) as dp,
            tc.tile_pool(name="yp", bufs=2) as yp,
            tc.tile_pool(name="pe", bufs=3) as pe,
            tc.tile_pool(name="rp", bufs=2) as rp,
            tc.tile_pool(name="op", bufs=3) as op,
            tc.tile_pool(name="scr", bufs=2) as scr,
            tc.tile_pool(name="psc", bufs=2, space="PSUM") as psc,
            tc.tile_pool(name="psnd", bufs=2, space="PSUM") as psnd,
        ):
            cstb = consts.tile([128, CWB], bf16, tag="cstb")
            nc.sync.dma_start(out=cstb, in_=CSTB[:])
            cstf = consts.tile([128, 5], f32, tag="cstf")
            nc.sync.dma_start(out=cstf, in_=CSTF[:])
            w1t = [cstb[:, 128 * dx: 128 * (dx + 1)] for dx in range(3)]
            w2t = [cstb[:, 384 + 128 * ab: 384 + 128 * (ab + 1)]
                   for ab in range(4)]
            bandT = [cstb[:, 896 + 128 * ab: 896 + 128 * (ab + 1)]
                     for ab in range(4)]
            b1t = cstf[:, 0:1]
            b2t = [cstf[:, 1 + ab: 2 + ab] for ab in range(4)]
            scrap0 = scr.tile([1, 1], f32, tag="scrap0")
            nc.vector.tensor_copy(scrap0, cstf[:1, :1])

            for b in range(nblocks):
                R = min(RB, HALF - RB * b)
                Rin = R + 2
                s = RB * b
                kp = R * 9   # partitions in (r,k) tiles
                yq = R * 8   # partitions in (r,o) tiles

                # --- load conv input block [(r,i), w] bf16 ---
                xb = xp.tile([128, WP], bf16, tag="xb")
                x_in = bass.AP(
                    tensor=X[:].tensor, offset=s * WP,
                    ap=[[WP, Rin], [(HALF + 2) * WP, C_IN], [1, WP]],
                )
                nc.sync.dma_start(out=xb[: Rin * 8], in_=x_in)

                # --- load unfolded depth [(r,k), x] bf16 (host-prepared) ---
                dunf = dp.tile([128, W], bf16, tag="dunf")
                nc.sync.dma_start(out=dunf, in_=DUNF[128 * b: 128 * (b + 1)])
                scrap = scr.tile([1, 1], bf16, tag="scrap")
                nc.vector.tensor_copy(scrap, dunf[:1, :1])  # eat DMA tick

                # --- conv1 + bias/relu per col chunk, on the shared 1-bank
                # conv pool (Y padded rows = relu(0+0)=0 so conv2 runs K=128)
                Y = yp.tile([128, W], bf16, tag="y")
                for c0, cn in ((0, 512), (512, 128)):
                    psum1 = psc.tile([128, 512], f32, tag="pscv")
                    for dx in range(3):
                        nc.tensor.matmul(
                            psum1[:, :cn],
                            w1t[dx][: Rin * 8],
                            xb[: Rin * 8, dx + c0: dx + c0 + cn],
                            start=(dx == 0), stop=(dx == 2),
                        )
                    nc.scalar.activation(
                        out=Y[:, c0:c0 + cn], in_=psum1[:, :cn],
                        func=mybir.ActivationFunctionType.Relu,
                        bias=b1t, scale=1.0,
                    )


                # --- conv2 + exp + P=E*d per ab; band-sum into psumND ---
                # psum2 is a single 1-bank [128,512] tile; conv2+exp run in
                # 512/128 col chunks so PSUM stays within 8 banks total.
                psumND = psnd.tile([128, 2 * W], f32, tag="psumnd")
                for ab in range(4):
                    PEt = pe.tile([128, 2 * W], bf16, tag="pet")
                    for c0, cn in ((0, 512), (512, 128)):
                        psum2 = psc.tile([128, 512], f32, tag="pscv")
                        nc.tensor.matmul(
                            psum2[:, :cn],
                            w2t[ab],
                            Y[:, c0:c0 + cn],
                            start=True, stop=True,
                        )
                        nc.scalar.activation(
                            out=PEt[:, W + c0: W + c0 + cn],
                            in_=psum2[:, :cn],
                            func=mybir.ActivationFunctionType.Exp,
                            bias=b2t[ab], scale=0.25,
                        )
                    nc.vector.tensor_mul(PEt[:, :W], PEt[:, W:], dunf)
                    # tap reduction: Num -> cols 0:640, Den -> cols 640:1280
                    for c0, cn in ((0, 512), (512, 512), (1024, 256)):
                        nc.tensor.matmul(
                            psumND[:, c0:c0 + cn],
                            bandT[ab],
                            PEt[:, c0:c0 + cn],
                            start=(ab == 0), stop=(ab == 3),
                        )

                # --- divide + b-interleave into OI (strided DVE writes) ---
                RD = rp.tile([128, W], f32, tag="rd")
                nc.vector.reciprocal_approx_fast(out=RD, in_=psumND[:, W:])
                OI = op.tile([128, 2 * W], f32, tag="oi")
                nc.vector.tensor_mul(
                    OI[0:78, 0:2 * W:2], psumND[0:78, :W], RD[0:78])
                # odd cols: quadrant reads can't span >32 partitions from
                # base 32, so two 14-partition ops (bases 32 and 96)
                nc.vector.tensor_mul(
                    OI[0:R, 1:2 * W:2], psumND[32:32 + R, :W], RD[32:32 + R])
                nc.vector.tensor_mul(
                    OI[64:64 + R, 1:2 * W:2], psumND[96:96 + R, :W],
                    RD[96:96 + R])

                # --- contiguous-row stores: out[2(s+r)+a, :] ---
                for a in range(2):
                    o_out = bass.AP(
                        tensor=OUT[:].tensor,
                        offset=(2 * s + a) * (2 * W),
                        ap=[[4 * W, R], [1, 2 * W]],
                    )
                    nc.sync.dma_start(out=o_out, in_=OI[64 * a: 64 * a + R])

    nc.compile()
    return nc


_NC_CACHE = None


def prep_inputs(depth, cost_volume, conv1_w, conv1_b, conv2_w, conv2_b):
    bf16 = ml_dtypes.bfloat16
    depth = np.asarray(depth, np.float32)
    cv = np.asarray(cost_volume, np.float32).reshape(N_IMG, C_IN, H, W)
    w1t, w2t, bandT, b1v, b2v = _build_consts(
        np.asarray(conv1_w, np.float32), np.asarray(conv1_b, np.float32),
        np.asarray(conv2_w, np.float32), np.asarray(conv2_b, np.float32))
    cstb, cstf = _pack_consts(w1t, w2t, bandT, b1v, b2v)

    # halo'd, zero-padded shards: core c = 2*n + h
    sw = np.lib.stride_tricks.sliding_window_view
    in_maps = []
    for n in range(N_IMG):
        cvp = np.zeros((C_IN, H + 2, WP), bf16)
        cvp[:, 1:H + 1, 1:W + 1] = cv[n]
        dpad = np.zeros((H + 2, WP), np.float32)
        dpad[1:H + 1, 1:W + 1] = depth[n]
        # unfold: du[(r*9 + ky*3 + kx), x] = dpad[r+ky, x+kx]
        win = sw(dpad, (3, W + 2))[:H, 0]                # [H,3,W+2]
        du = np.stack([win[:, :, kx:kx + W] for kx in range(3)], 2)
        du = du.reshape(H * 9, W).astype(bf16)
        nblk = (HALF + RB - 1) // RB
        for h in range(2):
            r0 = h * HALF
            # per-block 128-row pages: rows 9R..128 zero
            dup = np.zeros((nblk * 128, W), bf16)
            for blk in range(nblk):
                R = min(RB, HALF - RB * blk)
                src = 9 * (r0 + RB * blk)
                dup[128 * blk: 128 * blk + 9 * R] = du[src: src + 9 * R]
            in_maps.append({
                "xh": np.ascontiguousarray(cvp[:, r0:r0 + HALF + 2, :]),
                "dunf": dup,
                "cstb": cstb,
                "cstf": cstf,
            })
    return in_maps


def kernel(depth, cost_volume, conv1_w, conv1_b, conv2_w, conv2_b):
    global _NC_CACHE
    from concourse.bass_utils import run_bass_kernel_spmd

    in_maps = prep_inputs(depth, cost_volume, conv1_w, conv1_b,
                          conv2_w, conv2_b)
    if _NC_CACHE is None:
        _NC_CACHE = _build_bass()
    res = run_bass_kernel_spmd(_NC_CACHE, in_maps, core_ids=list(range(8)))
    out = np.empty((N_IMG, 2 * H, 2 * W), np.float32)
    for c, r in enumerate(res.results):
        n, h = c // 2, c % 2
        out[n, 2 * h * HALF: 2 * (h + 1) * HALF, :] = r["out"]
    return out
